# revision 2
# baseline (speedup 1.0000x reference)
"""Causal MHA (shared q_linear) Bass kernel for 8 TRN2 NeuronCores.

Sharding: core c handles batch b=c//2, head-group g=c%2 (8 of 16 heads,
columns 512g:512g+512 of the shared projection).  Each core computes a
partial output (its head-group's contribution through Wo); the host sums
the two partials per batch and adds bo.

Compute layout (per core, S=2048 tokens, D=1024, 8 heads of hd=64):
  xT  = transpose(x) via PE               [1024, 2048]  (fp32, exact)
  qT/kT = Wq_g^T @ xT (+bq)               [512, 2048]   (fp32r matmuls)
  v   = x @ Wq_g (+bq), stored [tok, head, 65] with a fused ones column
  scoresT[k,q] = kh @ qh^T (per head, K=64, two heads packed in PE rows)
  exp on ACT with scale=1/8, additive -1e10 causal mask on PSUM
  attnT[hd+1, q] = [vh|1]^T @ expT  accumulated over k in PSUM
     row 64 = sum(exp) -> reciprocal -> rank-1 PE broadcast -> normalize
  out = attnT^T @ Wo_g  (partial, host adds the two head-groups + bo)
"""

import sys

sys.path.insert(0, "/opt/trn_rl_repo")

import numpy as np
import concourse.bass as bass  # noqa: F401
import concourse.tile as tile
from concourse import bacc, mybir
from concourse.bass_utils import run_bass_kernel_spmd

F32 = mybir.dt.float32
F32R = mybir.dt.float32r
AF = mybir.ActivationFunctionType

S = 2048          # tokens
D = 1024          # model dim
DL = 512          # local (per-core) projection columns = 8 heads * 64
HD = 64           # head dim
NHL = 8           # local heads
TB = 4            # token blocks of 512
JD = 8            # Din blocks of 128
NEG = -1.0e10


def build(repeat: int = 1):
    nc = bacc.Bacc("TRN2", target_bir_lowering=False, debug=False)
    x_aps = {
        n: nc.dram_tensor(n, [S, D], F32, kind="ExternalInput").ap()
        for n in ("x_q", "x_k", "x_v")
    }
    wq_ap = nc.dram_tensor("wq", [D, DL], F32, kind="ExternalInput").ap()
    bq_ap = nc.dram_tensor("bq", [DL], F32, kind="ExternalInput").ap()
    wo_ap = nc.dram_tensor("wo", [DL, D], F32, kind="ExternalInput").ap()
    tri_ap = nc.dram_tensor("tri", [128, 128], F32, kind="ExternalInput").ap()
    id_ap = nc.dram_tensor("ident", [128, 128], F32, kind="ExternalInput").ap()
    out_ap = nc.dram_tensor("out", [S, D], F32, kind="ExternalOutput").ap()

    with tile.TileContext(nc) as tc:
        with tc.tile_pool(name="const", bufs=1) as const, \
             tc.tile_pool(name="persist", bufs=1) as persist, \
             tc.tile_pool(name="stage", bufs=2) as stage, \
             tc.tile_pool(name="xn", bufs=2) as xnp, \
             tc.tile_pool(name="xt", bufs=1) as xtp, \
             tc.tile_pool(name="qt", bufs=2) as qtp, \
             tc.tile_pool(name="exp", bufs=4) as ep, \
             tc.tile_pool(name="at", bufs=2) as atp, \
             tc.tile_pool(name="norm", bufs=2) as normp, \
             tc.tile_pool(name="ob", bufs=2) as obp, \
             tc.tile_pool(name="psA", bufs=4, space="PSUM") as psA, \
             tc.tile_pool(name="psAcc", bufs=2, space="PSUM") as psAcc:

            # ---- constants ----
            ident = const.tile([128, 128], F32)
            nc.sync.dma_start(ident[:], id_ap[:])
            tri = const.tile([128, 128], F32)
            nc.sync.dma_start(tri[:], tri_ap[:])
            bq_sb = const.tile([128, 4], F32)
            nc.sync.dma_start(bq_sb[:], bq_ap.rearrange("(t p) -> p t", p=128))
            bq_row = const.tile([1, DL], F32)
            nc.sync.dma_start(bq_row[:], bq_ap.rearrange("(a n) -> a n", a=1))
            bq_row_r = const.tile([1, DL], F32R)
            nc.vector.tensor_copy(bq_row_r[:], bq_row[:])
            ones_f = const.tile([128, 128], F32)
            nc.vector.memset(ones_f[:], 1.0)
            ones_r = const.tile([128, 128], F32R)
            nc.vector.tensor_copy(ones_r[:], ones_f[:])

            # ---- weights (cast to fp32r once) ----
            wq_r = persist.tile([128, JD, DL], F32R)
            for j in range(JD):
                st = stage.tile([128, DL], F32, tag="wst")
                nc.sync.dma_start(st[:], wq_ap[j * 128:(j + 1) * 128, :])
                nc.vector.tensor_copy(wq_r[:, j, :], st[:])
            wo_r = persist.tile([128, 4, D], F32R)
            for kt in range(4):
                st = stage.tile([128, D], F32, tag="wst2")
                nc.sync.dma_start(st[:], wo_ap[kt * 128:(kt + 1) * 128, :])
                nc.vector.tensor_copy(wo_r[:, kt, :], st[:])

            # persistent per-token-block tensors
            kT = [persist.tile([128, 4, 512], F32R, name=f"kT{i}", tag=f"kT{i}") for i in range(TB)]
            vv = [persist.tile([128, 4, NHL, HD + 1], F32R, name=f"vv{i}", tag=f"vv{i}") for i in range(TB)]

            q_tiles = [None] * TB

            def phase1(tb, rep):
                """transpose + project q,k,v for token block tb (512 tokens)."""
                for name in ("x_k", "x_v", "x_q"):
                    x_ap = x_aps[name]
                    xT = xtp.tile([128, JD, 512], F32R, tag="xt")
                    for sub in range(4):
                        xn = xnp.tile([128, D], F32, tag="xn")
                        r0 = tb * 512 + sub * 128
                        nc.sync.dma_start(xn[:], x_ap[r0:r0 + 128, :])
                        for j in range(JD):
                            pt = psA.tile([128, 128], F32, tag="mm")
                            nc.tensor.transpose(
                                pt[:], xn[:, j * 128:(j + 1) * 128], ident[:]
                            )
                            nc.vector.tensor_copy(
                                xT[:, j, sub * 128:(sub + 1) * 128], pt[:]
                            )
                    if name == "x_v":
                        vt = vv[tb]
                        for sub in range(4):
                            pv = psA.tile([128, 512], F32, tag="mm")
                            for j in range(JD):
                                nc.tensor.matmul(
                                    pv[:],
                                    xT[:, j, sub * 128:(sub + 1) * 128],
                                    wq_r[:, j, :],
                                    start=(j == 0),
                                    stop=False,
                                )
                            nc.tensor.matmul(
                                pv[:],
                                ones_r[0:1, 0:128],
                                bq_row_r[:],
                                start=False,
                                stop=True,
                            )
                            nc.vector.tensor_copy(
                                vt[:, sub, :, 0:HD],
                                pv[:].rearrange("p (h d) -> p h d", h=NHL),
                            )
                        nc.vector.tensor_copy(
                            vt[:, :, :, HD],
                            ones_f[:, 0:32].rearrange("p (s h) -> p s h", s=4),
                        )
                    else:
                        if name == "x_q":
                            dest = qtp.tile([128, 4, 512], F32R, tag="qt")
                            q_tiles[tb] = dest
                        else:
                            dest = kT[tb]
                        for dt_ in range(4):
                            py = psA.tile([128, 512], F32, tag="mm")
                            for j in range(JD):
                                nc.tensor.matmul(
                                    py[:],
                                    wq_r[:, j, dt_ * 128:(dt_ + 1) * 128],
                                    xT[:, j, :],
                                    start=(j == 0),
                                    stop=(j == JD - 1),
                                )
                            nc.scalar.activation(
                                dest[:, dt_, :],
                                py[:],
                                AF.Identity,
                                bias=bq_sb[:, dt_:dt_ + 1],
                            )

            def attention(Q, rep):
                """attention + Wo for query block Q (512 tokens)."""
                attnT = atp.tile([128, 4, 512], F32R, tag="at")
                qtile = q_tiles[Q]
                nj = 4 * (Q + 1)
                for hp in range(4):
                    acc0 = psAcc.tile([128, 512], F32, tag="acc")
                    acc1 = psAcc.tile([128, 512], F32, tag="acc")
                    for j in range(nj):
                        tbj, sub = j // 4, j % 4
                        qoff = max(0, j * 128 - Q * 512)
                        pss = []
                        for hi, base in ((0, 0), (1, 64)):
                            ps = psA.tile([128, 512], F32, tag="mm")
                            nc.tensor.matmul(
                                ps[:, qoff:],
                                kT[tbj][base:base + 64, hp,
                                        sub * 128:(sub + 1) * 128],
                                qtile[base:base + 64, hp, qoff:],
                                start=True,
                                stop=True,
                            )
                            pss.append(ps)
                        diag = j * 128 >= Q * 512
                        for hi, (ps, acc) in enumerate(((pss[0], acc0),
                                                        (pss[1], acc1))):
                            if diag:
                                nc.vector.tensor_add(
                                    ps[:, qoff:qoff + 128],
                                    ps[:, qoff:qoff + 128],
                                    tri[:],
                                )
                            et = ep.tile([128, 512], F32R, tag="exp")
                            nc.scalar.activation(
                                et[:, qoff:], ps[:, qoff:], AF.Exp, scale=0.125
                            )
                            nc.tensor.matmul(
                                acc[0:65, qoff:],
                                vv[tbj][:, sub, hp * 2 + hi, :],
                                et[:, qoff:],
                                start=(j == 0),
                                stop=(j == nj - 1),
                            )
                    for hi, acc in ((0, acc0), (1, acc1)):
                        rf = normp.tile([128, 512], F32, tag="rf")
                        nc.vector.reciprocal(rf[64:65, :], acc[64:65, :])
                        rr = normp.tile([128, 512], F32R, tag="rr")
                        nc.vector.tensor_copy(rr[64:65, :], rf[64:65, :])
                        pb = psA.tile([128, 512], F32, tag="mm")
                        nc.tensor.matmul(
                            pb[0:64, :],
                            ones_r[64:65, 0:64],
                            rr[64:65, :],
                            start=True,
                            stop=True,
                        )
                        bs = normp.tile([64, 512], F32, tag="bs")
                        nc.scalar.copy(bs[:], pb[0:64, :])
                        nc.vector.tensor_mul(
                            attnT[hi * 64:(hi + 1) * 64, hp, :],
                            acc[0:64, :],
                            bs[:],
                        )
                # Wo projection for this token block
                for st_ in range(4):
                    ob = obp.tile([128, D], F32, tag="ob")
                    for nh in range(2):
                        po = psA.tile([128, 512], F32, tag="mm")
                        for kt in range(4):
                            nc.tensor.matmul(
                                po[:],
                                attnT[:, kt, st_ * 128:(st_ + 1) * 128],
                                wo_r[:, kt, nh * 512:(nh + 1) * 512],
                                start=(kt == 0),
                                stop=(kt == 3),
                            )
                        nc.vector.tensor_copy(ob[:, nh * 512:(nh + 1) * 512], po[:])
                    r0 = Q * 512 + st_ * 128
                    nc.sync.dma_start(out_ap[r0:r0 + 128, :], ob[:])

            for rep in range(repeat):
                phase1(0, rep)
                phase1(1, rep)
                attention(0, rep)
                phase1(2, rep)
                attention(1, rep)
                phase1(3, rep)
                attention(2, rep)
                attention(3, rep)

    nc.compile()
    return nc


_BUILD_CACHE = {}


def _get(repeat=1):
    if repeat not in _BUILD_CACHE:
        _BUILD_CACHE[repeat] = build(repeat)
    return _BUILD_CACHE[repeat]


def make_in_maps(q, k, v, Wq, bq, Wo, bo):
    tri = np.where(
        np.arange(128)[:, None] <= np.arange(128)[None, :], 0.0, NEG
    ).astype(np.float32)
    ident = np.eye(128, dtype=np.float32)
    in_maps = []
    for c in range(8):
        b, g = c // 2, c % 2
        sl = slice(g * DL, (g + 1) * DL)
        in_maps.append({
            "x_q": np.ascontiguousarray(q[b]),
            "x_k": np.ascontiguousarray(k[b]),
            "x_v": np.ascontiguousarray(v[b]),
            "wq": np.ascontiguousarray(Wq[:, sl]),
            "bq": np.ascontiguousarray(bq[sl]),
            "wo": np.ascontiguousarray(Wo[sl, :]),
            "tri": tri,
            "ident": ident,
        })
    return in_maps


def kernel(q, k, v, Wq, bq, Wo, bo):
    q, k, v, Wq, bq, Wo, bo = (
        np.asarray(a, dtype=np.float32) for a in (q, k, v, Wq, bq, Wo, bo)
    )
    nc = _get(1)
    in_maps = make_in_maps(q, k, v, Wq, bq, Wo, bo)
    res = run_bass_kernel_spmd(nc, in_maps, list(range(8)))
    B = q.shape[0]
    out = np.empty((B, S, D), dtype=np.float32)
    for b in range(B):
        out[b] = res.results[2 * b]["out"] + res.results[2 * b + 1]["out"] + bo
    return out


# revision 7
# speedup vs baseline: 1.0747x; 1.0747x over previous
"""Causal MHA (shared q_linear) Bass kernel for 8 TRN2 NeuronCores.

Sharding: core c handles batch b=c//2, head-group g=c%2 (8 of 16 heads,
columns 512g:512g+512 of the shared projection).  Each core computes a
partial output (its head-group's contribution through Wo); the host sums
the two partials per batch and adds bo.

Compute layout (per core, S=2048 tokens, D=1024, 8 heads of hd=64):
  xT  = transpose(x) via PE               [1024, 2048]  (fp32, exact)
  qT/kT = Wq_g^T @ xT (+bq)               [512, 2048]   (fp32r matmuls)
  v   = x @ Wq_g (+bq), stored [tok, head, 65] with a fused ones column
  scoresT[k,q] = kh @ qh^T (per head, K=64, two heads packed in PE rows)
  exp on ACT with scale=1/8, additive -1e10 causal mask on PSUM
  attnT[hd+1, q] = [vh|1]^T @ expT  accumulated over k in PSUM
     row 64 = sum(exp) -> reciprocal -> rank-1 PE broadcast -> normalize
  out = attnT^T @ Wo_g  (partial, host adds the two head-groups + bo)
"""

import sys

sys.path.insert(0, "/opt/trn_rl_repo")

import numpy as np
import concourse.bass as bass  # noqa: F401
import concourse.tile as tile
from concourse import bacc, mybir
from concourse.bass_utils import run_bass_kernel_spmd

F32 = mybir.dt.float32
F32R = mybir.dt.float32r
AF = mybir.ActivationFunctionType

S = 2048          # tokens
D = 1024          # model dim
DL = 512          # local (per-core) projection columns = 8 heads * 64
HD = 64           # head dim
NHL = 8           # local heads
TB = 4            # token blocks of 512
JD = 8            # Din blocks of 128
NEG = -1.0e10


def build(repeat: int = 1):
    nc = bacc.Bacc("TRN2", target_bir_lowering=False, debug=False)
    x_aps = {
        n: nc.dram_tensor(n, [S, D], F32, kind="ExternalInput").ap()
        for n in ("x_q", "x_k", "x_v")
    }
    wq_ap = nc.dram_tensor("wq", [D, DL], F32, kind="ExternalInput").ap()
    bq_ap = nc.dram_tensor("bq", [DL], F32, kind="ExternalInput").ap()
    wo_ap = nc.dram_tensor("wo", [DL, D], F32, kind="ExternalInput").ap()
    tri_ap = nc.dram_tensor("tri", [128, 128], F32, kind="ExternalInput").ap()
    id_ap = nc.dram_tensor("ident", [128, 128], F32, kind="ExternalInput").ap()
    out_ap = nc.dram_tensor("out", [S, D], F32, kind="ExternalOutput").ap()

    with tile.TileContext(nc) as tc:
        with tc.tile_pool(name="const", bufs=1) as const, \
             tc.tile_pool(name="persist", bufs=1) as persist, \
             tc.tile_pool(name="stage", bufs=1) as stage, \
             tc.tile_pool(name="xn", bufs=2) as xnp, \
             tc.tile_pool(name="xt", bufs=1) as xtp, \
             tc.tile_pool(name="qt", bufs=4) as qtp, \
             tc.tile_pool(name="exp", bufs=2) as ep, \
             tc.tile_pool(name="at", bufs=2) as atp, \
             tc.tile_pool(name="norm", bufs=2) as normp, \
             tc.tile_pool(name="ob", bufs=2) as obp, \
             tc.tile_pool(name="psA", bufs=2, space="PSUM") as psA, \
             tc.tile_pool(name="psS", bufs=2, space="PSUM") as psS, \
             tc.tile_pool(name="psAcc", bufs=2, space="PSUM") as psAcc:

            # ---- constants ----
            ident = const.tile([128, 128], F32)
            nc.sync.dma_start(ident[:], id_ap[:])
            tri = const.tile([128, 128], F32)
            nc.sync.dma_start(tri[:], tri_ap[:])
            bq_sb = const.tile([128, 4], F32)
            nc.sync.dma_start(bq_sb[:], bq_ap.rearrange("(t p) -> p t", p=128))
            bq_row = const.tile([1, DL], F32)
            nc.sync.dma_start(bq_row[:], bq_ap.rearrange("(a n) -> a n", a=1))
            bq_row_r = const.tile([1, DL], F32R)
            nc.vector.tensor_copy(bq_row_r[:], bq_row[:])
            ones_f = const.tile([128, 128], F32)
            nc.vector.memset(ones_f[:], 1.0)
            ones_r = const.tile([128, 128], F32R)
            nc.vector.tensor_copy(ones_r[:], ones_f[:])

            # ---- weights (cast to fp32r once) ----
            wq_r = persist.tile([128, JD, DL], F32R)
            for j in range(JD):
                st = stage.tile([128, DL], F32, tag="wst")
                nc.sync.dma_start(st[:], wq_ap[j * 128:(j + 1) * 128, :])
                nc.vector.tensor_copy(wq_r[:, j, :], st[:])
            wo_r = persist.tile([128, 4, D], F32R)
            for kt in range(4):
                st = stage.tile([128, D], F32, tag="wst2")
                nc.sync.dma_start(st[:], wo_ap[kt * 128:(kt + 1) * 128, :])
                nc.vector.tensor_copy(wo_r[:, kt, :], st[:])

            # persistent per-token-block tensors
            kT = [persist.tile([128, 4, 512], F32R, name=f"kT{i}", tag=f"kT{i}") for i in range(TB)]
            vv = [persist.tile([128, 4, NHL, HD + 1], F32R, name=f"vv{i}", tag=f"vv{i}") for i in range(TB)]

            q_tiles = [None] * TB

            def phase1(tb, rep):
                """transpose + project q,k,v for token block tb (512 tokens)."""
                for name in ("x_k", "x_v", "x_q"):
                    x_ap = x_aps[name]
                    xT = xtp.tile([128, JD, 512], F32R, tag="xt")
                    for sub in range(4):
                        xn = xnp.tile([128, D], F32, tag="xn")
                        r0 = tb * 512 + sub * 128
                        nc.sync.dma_start(xn[:], x_ap[r0:r0 + 128, :])
                        for jg in range(2):
                            pt = psA.tile([128, 512], F32, tag="mm")
                            for ji in range(4):
                                j = jg * 4 + ji
                                nc.tensor.transpose(
                                    pt[:, ji * 128:(ji + 1) * 128],
                                    xn[:, j * 128:(j + 1) * 128],
                                    ident[:],
                                )
                            nc.vector.tensor_copy(
                                xT[:, jg * 4:(jg + 1) * 4,
                                   sub * 128:(sub + 1) * 128],
                                pt[:].rearrange("p (j t) -> p j t", j=4),
                            )
                    if name == "x_v":
                        vt = vv[tb]
                        for sub in range(4):
                            pv = psA.tile([128, 512], F32, tag="mm")
                            for j in range(JD):
                                nc.tensor.matmul(
                                    pv[:],
                                    xT[:, j, sub * 128:(sub + 1) * 128],
                                    wq_r[:, j, :],
                                    start=(j == 0),
                                    stop=False,
                                )
                            nc.tensor.matmul(
                                pv[:],
                                ones_r[0:1, 0:128],
                                bq_row_r[:],
                                start=False,
                                stop=True,
                            )
                            nc.vector.tensor_copy(
                                vt[:, sub, :, 0:HD],
                                pv[:].rearrange("p (h d) -> p h d", h=NHL),
                            )
                        nc.vector.tensor_copy(
                            vt[:, :, :, HD],
                            ones_f[:, 0:32].rearrange("p (s h) -> p s h", s=4),
                        )
                    else:
                        if name == "x_q":
                            dest = qtp.tile([128, 4, 512], F32R, tag="qt")
                            q_tiles[tb] = dest
                        else:
                            dest = kT[tb]
                        for dt_ in range(4):
                            py = psA.tile([128, 512], F32, tag="mm")
                            for j in range(JD):
                                nc.tensor.matmul(
                                    py[:],
                                    wq_r[:, j, dt_ * 128:(dt_ + 1) * 128],
                                    xT[:, j, :],
                                    start=(j == 0),
                                    stop=(j == JD - 1),
                                )
                            nc.scalar.activation(
                                dest[:, dt_, :],
                                py[:],
                                AF.Identity,
                                bias=bq_sb[:, dt_:dt_ + 1],
                            )

            def attention(Q, rep):
                """attention + Wo for query block Q (512 tokens)."""
                attnT = atp.tile([128, 4, 512], F32R, tag="at")
                qtile = q_tiles[Q]
                nj = 4 * (Q + 1)
                for hp in range(4):
                    acc0 = psAcc.tile([128, 512], F32, tag="acc")
                    acc1 = psAcc.tile([128, 512], F32, tag="acc")
                    for j in range(nj):
                        tbj, sub = j // 4, j % 4
                        qoff = max(0, j * 128 - Q * 512)
                        # both heads' scoresT tiles side by side in one
                        # 2-bank PSUM tile -> single exp instruction
                        ps = psS.tile([128, 2, 512], F32, tag="sc")
                        for hi, base in ((0, 0), (1, 64)):
                            nc.tensor.matmul(
                                ps[:, hi, qoff:],
                                kT[tbj][base:base + 64, hp,
                                        sub * 128:(sub + 1) * 128],
                                qtile[base:base + 64, hp, qoff:],
                                start=True,
                                stop=True,
                            )
                        if j * 128 >= Q * 512:  # diagonal k-tile: causal mask
                            for hi in range(2):
                                nc.vector.tensor_add(
                                    ps[:, hi, qoff:qoff + 128],
                                    ps[:, hi, qoff:qoff + 128],
                                    tri[:],
                                )
                        et = ep.tile([128, 2, 512], F32R, tag="exp")
                        nc.scalar.activation(
                            et[:, :, qoff:], ps[:, :, qoff:], AF.Exp, scale=0.125
                        )
                        for hi, acc in ((0, acc0), (1, acc1)):
                            nc.tensor.matmul(
                                acc[0:65, qoff:],
                                vv[tbj][:, sub, hp * 2 + hi, :],
                                et[:, hi, qoff:],
                                start=(j == 0),
                                stop=(j == nj - 1),
                            )
                    for hi, acc in ((0, acc0), (1, acc1)):
                        # sum row (f32r) -> rank-1 broadcast -> wide recip
                        sr = normp.tile([128, 512], F32R, tag="sr")
                        nc.vector.tensor_copy(sr[64:65, :], acc[64:65, :])
                        pb = psA.tile([128, 512], F32, tag="mm")
                        nc.tensor.matmul(
                            pb[0:64, :],
                            ones_r[64:65, 0:64],
                            sr[64:65, :],
                            start=True,
                            stop=True,
                        )
                        rb = normp.tile([64, 512], F32, tag="rb")
                        nc.vector.reciprocal(rb[:], pb[0:64, :])
                        nc.vector.tensor_mul(
                            attnT[hi * 64:(hi + 1) * 64, hp, :],
                            acc[0:64, :],
                            rb[:],
                        )
                # Wo projection for this token block
                for st_ in range(4):
                    ob = obp.tile([128, D], F32, tag="ob")
                    for nh in range(2):
                        po = psA.tile([128, 512], F32, tag="mm")
                        for kt in range(4):
                            nc.tensor.matmul(
                                po[:],
                                attnT[:, kt, st_ * 128:(st_ + 1) * 128],
                                wo_r[:, kt, nh * 512:(nh + 1) * 512],
                                start=(kt == 0),
                                stop=(kt == 3),
                            )
                        nc.vector.tensor_copy(ob[:, nh * 512:(nh + 1) * 512], po[:])
                    r0 = Q * 512 + st_ * 128
                    nc.sync.dma_start(out_ap[r0:r0 + 128, :], ob[:])

            for rep in range(repeat):
                for tb in range(TB):
                    phase1(tb, rep)
                for Q in range(TB):
                    attention(Q, rep)

    nc.compile()
    return nc


_BUILD_CACHE = {}


def _get(repeat=1):
    if repeat not in _BUILD_CACHE:
        _BUILD_CACHE[repeat] = build(repeat)
    return _BUILD_CACHE[repeat]


def make_in_maps(q, k, v, Wq, bq, Wo, bo):
    tri = np.where(
        np.arange(128)[:, None] <= np.arange(128)[None, :], 0.0, NEG
    ).astype(np.float32)
    ident = np.eye(128, dtype=np.float32)
    in_maps = []
    for c in range(8):
        b, g = c // 2, c % 2
        sl = slice(g * DL, (g + 1) * DL)
        in_maps.append({
            "x_q": np.ascontiguousarray(q[b]),
            "x_k": np.ascontiguousarray(k[b]),
            "x_v": np.ascontiguousarray(v[b]),
            "wq": np.ascontiguousarray(Wq[:, sl]),
            "bq": np.ascontiguousarray(bq[sl]),
            "wo": np.ascontiguousarray(Wo[sl, :]),
            "tri": tri,
            "ident": ident,
        })
    return in_maps


def kernel(q, k, v, Wq, bq, Wo, bo):
    q, k, v, Wq, bq, Wo, bo = (
        np.asarray(a, dtype=np.float32) for a in (q, k, v, Wq, bq, Wo, bo)
    )
    nc = _get(1)
    in_maps = make_in_maps(q, k, v, Wq, bq, Wo, bo)
    res = run_bass_kernel_spmd(nc, in_maps, list(range(8)))
    B = q.shape[0]
    out = np.empty((B, S, D), dtype=np.float32)
    for b in range(B):
        out[b] = res.results[2 * b]["out"] + res.results[2 * b + 1]["out"] + bo
    return out


# revision 10
# speedup vs baseline: 1.6865x; 1.5693x over previous
"""Causal MHA (shared q_linear) Bass kernel for 8 TRN2 NeuronCores.

Sharding: core c handles batch b=c//2, head-group g=c%2 (8 of 16 heads,
columns 512g:512g+512 of the shared projection).  Each core computes a
partial output (its head-group's contribution through Wo); the host sums
the two partials per batch and adds bo.

Compute layout (per core, S=2048 tokens, D=1024, 8 heads of hd=64):
  xT  = transpose(x) via PE               [1024, 2048]  (fp32, exact)
  qT/kT = Wq_g^T @ xT (+bq)               [512, 2048]   (fp32r matmuls)
  v   = x @ Wq_g (+bq), stored [tok, head, 65] with a fused ones column
  scoresT[k,q] = kh @ qh^T (per head, K=64, two heads packed in PE rows)
  exp on ACT with scale=1/8, additive -1e10 causal mask on PSUM
  attnT[hd+1, q] = [vh|1]^T @ expT  accumulated over k in PSUM
     row 64 = sum(exp) -> reciprocal -> rank-1 PE broadcast -> normalize
  out = attnT^T @ Wo_g  (partial, host adds the two head-groups + bo)
"""

import sys

sys.path.insert(0, "/opt/trn_rl_repo")

import numpy as np
import concourse.bass as bass  # noqa: F401
import concourse.tile as tile
from concourse import bacc, mybir
from concourse.bass_utils import run_bass_kernel_spmd

F32 = mybir.dt.float32
F32R = mybir.dt.float32r
AF = mybir.ActivationFunctionType

S = 2048          # tokens
D = 1024          # model dim
DL = 512          # local (per-core) projection columns = 8 heads * 64
HD = 64           # head dim
NHL = 8           # local heads
TB = 4            # token blocks of 512
JD = 8            # Din blocks of 128
NEG = -1.0e10


def build(repeat: int = 1, mode: str = "full"):
    nc = bacc.Bacc("TRN2", target_bir_lowering=False, debug=False)
    x_aps = {
        n: nc.dram_tensor(n, [S, D], F32, kind="ExternalInput").ap()
        for n in ("x_q", "x_k", "x_v")
    }
    wq_ap = nc.dram_tensor("wq", [D, DL], F32, kind="ExternalInput").ap()
    bq_ap = nc.dram_tensor("bq", [DL], F32, kind="ExternalInput").ap()
    wo_ap = nc.dram_tensor("wo", [DL, D], F32, kind="ExternalInput").ap()
    tri_ap = nc.dram_tensor("tri", [128, 128], F32, kind="ExternalInput").ap()
    id_ap = nc.dram_tensor("ident", [128, 128], F32, kind="ExternalInput").ap()
    out_ap = nc.dram_tensor("out", [S, D], F32, kind="ExternalOutput").ap()

    with tile.TileContext(nc) as tc:
        with tc.tile_pool(name="const", bufs=1) as const, \
             tc.tile_pool(name="persist", bufs=1) as persist, \
             tc.tile_pool(name="stage", bufs=1) as stage, \
             tc.tile_pool(name="xn", bufs=2) as xnp, \
             tc.tile_pool(name="xt", bufs=1) as xtp, \
             tc.tile_pool(name="qt", bufs=4) as qtp, \
             tc.tile_pool(name="exp", bufs=2) as ep, \
             tc.tile_pool(name="at", bufs=2) as atp, \
             tc.tile_pool(name="norm", bufs=2) as normp, \
             tc.tile_pool(name="ob", bufs=2) as obp, \
             tc.tile_pool(name="psA", bufs=2, space="PSUM") as psA, \
             tc.tile_pool(name="psS", bufs=2, space="PSUM") as psS, \
             tc.tile_pool(name="psAcc", bufs=2, space="PSUM") as psAcc:

            # ---- constants ----
            ident = const.tile([128, 128], F32)
            nc.sync.dma_start(ident[:], id_ap[:])
            tri = const.tile([128, 128], F32)
            nc.sync.dma_start(tri[:], tri_ap[:])
            bq_sb = const.tile([128, 4], F32)
            nc.sync.dma_start(bq_sb[:], bq_ap.rearrange("(t p) -> p t", p=128))
            bq_row = const.tile([1, DL], F32)
            nc.sync.dma_start(bq_row[:], bq_ap.rearrange("(a n) -> a n", a=1))
            bq_row_r = const.tile([1, DL], F32R)
            nc.vector.tensor_copy(bq_row_r[:], bq_row[:])
            ones_f = const.tile([128, 128], F32)
            nc.vector.memset(ones_f[:], 1.0)
            ones_r = const.tile([128, 128], F32R)
            nc.vector.tensor_copy(ones_r[:], ones_f[:])

            # ---- weights (cast to fp32r once) ----
            wq_r = persist.tile([128, JD, DL], F32R)
            for j in range(JD):
                st = stage.tile([128, DL], F32, tag="wst")
                nc.sync.dma_start(st[:], wq_ap[j * 128:(j + 1) * 128, :])
                nc.vector.tensor_copy(wq_r[:, j, :], st[:])
            wo_r = persist.tile([128, 4, D], F32R)
            for kt in range(4):
                st = stage.tile([128, D], F32, tag="wst2")
                nc.sync.dma_start(st[:], wo_ap[kt * 128:(kt + 1) * 128, :])
                nc.vector.tensor_copy(wo_r[:, kt, :], st[:])

            # persistent per-token-block tensors
            kT = [persist.tile([128, 4, 512], F32R, name=f"kT{i}", tag=f"kT{i}") for i in range(TB)]
            vv = [persist.tile([128, 4, NHL, HD + 1], F32R, name=f"vv{i}", tag=f"vv{i}") for i in range(TB)]

            q_tiles = [None] * TB

            def phase1(tb, rep):
                """transpose + project q,k,v for token block tb (512 tokens)."""
                for name in ("x_k", "x_v", "x_q"):
                    x_ap = x_aps[name]
                    xT = xtp.tile([128, JD, 512], F32R, tag="xt")
                    for sub in range(4):
                        xn = xnp.tile([128, D], F32, tag="xn")
                        r0 = tb * 512 + sub * 128
                        nc.sync.dma_start(xn[:], x_ap[r0:r0 + 128, :])
                        for jg in range(2):
                            pt = psA.tile([128, 512], F32, tag="mm")
                            for ji in range(4):
                                j = jg * 4 + ji
                                nc.tensor.transpose(
                                    pt[:, ji * 128:(ji + 1) * 128],
                                    xn[:, j * 128:(j + 1) * 128],
                                    ident[:],
                                )
                            nc.vector.tensor_copy(
                                xT[:, jg * 4:(jg + 1) * 4,
                                   sub * 128:(sub + 1) * 128],
                                pt[:].rearrange("p (j t) -> p j t", j=4),
                            )
                    if name == "x_v":
                        vt = vv[tb]
                        for sub in range(4):
                            pv = psA.tile([128, 512], F32, tag="mm")
                            for j in range(JD):
                                nc.tensor.matmul(
                                    pv[:],
                                    xT[:, j, sub * 128:(sub + 1) * 128],
                                    wq_r[:, j, :],
                                    start=(j == 0),
                                    stop=False,
                                )
                            nc.tensor.matmul(
                                pv[:],
                                ones_r[0:1, 0:128],
                                bq_row_r[:],
                                start=False,
                                stop=True,
                            )
                            nc.vector.tensor_copy(
                                vt[:, sub, :, 0:HD],
                                pv[:].rearrange("p (h d) -> p h d", h=NHL),
                            )
                        nc.vector.tensor_copy(
                            vt[:, :, :, HD],
                            ones_f[:, 0:32].rearrange("p (s h) -> p s h", s=4),
                        )
                    else:
                        if name == "x_q":
                            dest = qtp.tile([128, 4, 512], F32R, tag="qt")
                            q_tiles[tb] = dest
                        else:
                            dest = kT[tb]
                        for dt_ in range(4):
                            py = psA.tile([128, 512], F32, tag="mm")
                            for j in range(JD):
                                nc.tensor.matmul(
                                    py[:],
                                    wq_r[:, j, dt_ * 128:(dt_ + 1) * 128],
                                    xT[:, j, :],
                                    start=(j == 0),
                                    stop=(j == JD - 1),
                                )
                            nc.scalar.activation(
                                dest[:, dt_, :],
                                py[:],
                                AF.Identity,
                                bias=bq_sb[:, dt_:dt_ + 1],
                            )

            def attention(Q, rep):
                """attention + Wo for query block Q (512 tokens)."""
                attnT = atp.tile([128, 4, 512], F32R, tag="at")
                qtile = q_tiles[Q]
                nj = 4 * (Q + 1)
                for hp in range(4):
                    acc0 = psAcc.tile([128, 512], F32, tag="acc")
                    acc1 = psAcc.tile([128, 512], F32, tag="acc")
                    for j in range(nj):
                        tbj, sub = j // 4, j % 4
                        qoff = max(0, j * 128 - Q * 512)
                        # both heads' scoresT tiles side by side in one
                        # 2-bank PSUM tile -> single exp instruction
                        ps = psS.tile([128, 2, 512], F32, tag="sc")
                        for hi, base in ((0, 0), (1, 64)):
                            nc.tensor.matmul(
                                ps[:, hi, qoff:],
                                kT[tbj][base:base + 64, hp,
                                        sub * 128:(sub + 1) * 128],
                                qtile[base:base + 64, hp, qoff:],
                                start=True,
                                stop=True,
                            )
                        if j * 128 >= Q * 512:  # diagonal k-tile: causal mask
                            for hi in range(2):
                                nc.vector.tensor_add(
                                    ps[:, hi, qoff:qoff + 128],
                                    ps[:, hi, qoff:qoff + 128],
                                    tri[:],
                                )
                        et = ep.tile([128, 2, 512], F32R, tag="exp")
                        nc.scalar.activation(
                            et[:, :, qoff:], ps[:, :, qoff:], AF.Exp, scale=0.125
                        )
                        for hi, acc in ((0, acc0), (1, acc1)):
                            nc.tensor.matmul(
                                acc[0:65, qoff:],
                                vv[tbj][:, sub, hp * 2 + hi, :],
                                et[:, hi, qoff:],
                                start=(j == 0),
                                stop=(j == nj - 1),
                            )
                    for hi, acc in ((0, acc0), (1, acc1)):
                        # sum row (f32r) -> rank-1 broadcast -> wide recip
                        sr = normp.tile([128, 512], F32R, tag="sr")
                        nc.vector.tensor_copy(sr[64:65, :], acc[64:65, :])
                        pb = psA.tile([128, 512], F32, tag="mm")
                        nc.tensor.matmul(
                            pb[0:64, :],
                            ones_r[64:65, 0:64],
                            sr[64:65, :],
                            start=True,
                            stop=True,
                        )
                        rb = normp.tile([64, 512], F32, tag="rb")
                        nc.vector.reciprocal(rb[:], pb[0:64, :])
                        nc.vector.tensor_mul(
                            attnT[hi * 64:(hi + 1) * 64, hp, :],
                            acc[0:64, :],
                            rb[:],
                        )
                # Wo projection for this token block
                for st_ in range(4):
                    ob = obp.tile([128, D], F32, tag="ob")
                    for nh in range(2):
                        po = psA.tile([128, 512], F32, tag="mm")
                        for kt in range(4):
                            nc.tensor.matmul(
                                po[:],
                                attnT[:, kt, st_ * 128:(st_ + 1) * 128],
                                wo_r[:, kt, nh * 512:(nh + 1) * 512],
                                start=(kt == 0),
                                stop=(kt == 3),
                            )
                        nc.vector.tensor_copy(ob[:, nh * 512:(nh + 1) * 512], po[:])
                    r0 = Q * 512 + st_ * 128
                    nc.sync.dma_start(out_ap[r0:r0 + 128, :], ob[:])

            if mode == "full":
                for rep in range(repeat):
                    for tb in range(TB):
                        phase1(tb, rep)
                    for Q in range(TB):
                        attention(Q, rep)
            elif mode == "p1":
                for rep in range(repeat):
                    for tb in range(TB):
                        phase1(tb, rep)
                for Q in range(TB):
                    attention(Q, 0)
            elif mode == "attn":
                for tb in range(TB):
                    phase1(tb, 0)
                for rep in range(repeat):
                    for Q in range(TB):
                        attention(Q, rep)

    nc.compile()
    return nc


_BUILD_CACHE = {}


def _get(repeat=1, mode="full"):
    key = (repeat, mode)
    if key not in _BUILD_CACHE:
        _BUILD_CACHE[key] = build(repeat, mode)
    return _BUILD_CACHE[key]


def make_in_maps(q, k, v, Wq, bq, Wo, bo):
    tri = np.where(
        np.arange(128)[:, None] <= np.arange(128)[None, :], 0.0, NEG
    ).astype(np.float32)
    ident = np.eye(128, dtype=np.float32)
    in_maps = []
    for c in range(8):
        b, g = c // 2, c % 2
        sl = slice(g * DL, (g + 1) * DL)
        in_maps.append({
            "x_q": np.ascontiguousarray(q[b]),
            "x_k": np.ascontiguousarray(k[b]),
            "x_v": np.ascontiguousarray(v[b]),
            "wq": np.ascontiguousarray(Wq[:, sl]),
            "bq": np.ascontiguousarray(bq[sl]),
            "wo": np.ascontiguousarray(Wo[sl, :]),
            "tri": tri,
            "ident": ident,
        })
    return in_maps


def kernel(q, k, v, Wq, bq, Wo, bo):
    q, k, v, Wq, bq, Wo, bo = (
        np.asarray(a, dtype=np.float32) for a in (q, k, v, Wq, bq, Wo, bo)
    )
    nc = _get(1)
    in_maps = make_in_maps(q, k, v, Wq, bq, Wo, bo)
    res = run_bass_kernel_spmd(nc, in_maps, list(range(8)))
    B = q.shape[0]
    out = np.empty((B, S, D), dtype=np.float32)
    for b in range(B):
        out[b] = res.results[2 * b]["out"] + res.results[2 * b + 1]["out"] + bo
    return out


# revision 13
# speedup vs baseline: 2.9308x; 1.7377x over previous
"""Causal MHA (shared q_linear) Bass kernel for 8 TRN2 NeuronCores.

Sharding: core c handles batch b=c//2, head-group g=c%2 (8 of 16 heads,
columns 512g:512g+512 of the shared projection).  Each core computes a
partial output (its head-group's contribution through Wo); the host sums
the two partials per batch and adds bo.

Compute layout (per core, S=2048 tokens, D=1024, 8 heads of hd=64):
  xT  = transpose(x) via PE               [1024, 2048]  (fp32, exact)
  qT/kT = Wq_g^T @ xT (+bq)               [512, 2048]   (fp32r matmuls)
  v   = x @ Wq_g (+bq), stored [tok, head, 65] with a fused ones column
  scoresT[k,q] = kh @ qh^T (per head, K=64, two heads packed in PE rows)
  exp on ACT with scale=1/8, additive -1e10 causal mask on PSUM
  attnT[hd+1, q] = [vh|1]^T @ expT  accumulated over k in PSUM
     row 64 = sum(exp) -> reciprocal -> rank-1 PE broadcast -> normalize
  out = attnT^T @ Wo_g  (partial, host adds the two head-groups + bo)
"""

import sys

sys.path.insert(0, "/opt/trn_rl_repo")

import numpy as np
import concourse.bass as bass  # noqa: F401
import concourse.tile as tile
from concourse import bacc, mybir
from concourse.bass_utils import run_bass_kernel_spmd

F32 = mybir.dt.float32
F32R = mybir.dt.float32r
AF = mybir.ActivationFunctionType

S = 2048          # tokens
D = 1024          # model dim
DL = 512          # local (per-core) projection columns = 8 heads * 64
HD = 64           # head dim
NHL = 8           # local heads
TB = 4            # token blocks of 512
JD = 8            # Din blocks of 128
NEG = -1.0e10


def build(repeat: int = 1, mode: str = "full"):
    nc = bacc.Bacc("TRN2", target_bir_lowering=False, debug=False)
    x_aps = {
        n: nc.dram_tensor(n, [S, D], F32, kind="ExternalInput").ap()
        for n in ("x_q", "x_k", "x_v")
    }
    wq_ap = nc.dram_tensor("wq", [D, DL], F32, kind="ExternalInput").ap()
    bq_ap = nc.dram_tensor("bq", [DL], F32, kind="ExternalInput").ap()
    wo_ap = nc.dram_tensor("wo", [DL, D], F32, kind="ExternalInput").ap()
    tri_ap = nc.dram_tensor("tri", [128, 128], F32, kind="ExternalInput").ap()
    id_ap = nc.dram_tensor("ident", [128, 128], F32, kind="ExternalInput").ap()
    out_ap = nc.dram_tensor("out", [S, D], F32, kind="ExternalOutput").ap()

    with tile.TileContext(nc) as tc:
        with tc.tile_pool(name="const", bufs=1) as const, \
             tc.tile_pool(name="persist", bufs=1) as persist, \
             tc.tile_pool(name="stage", bufs=1) as stage, \
             tc.tile_pool(name="xn", bufs=2) as xnp, \
             tc.tile_pool(name="xt", bufs=1) as xtp, \
             tc.tile_pool(name="qt", bufs=4) as qtp, \
             tc.tile_pool(name="exp", bufs=2) as ep, \
             tc.tile_pool(name="at", bufs=2) as atp, \
             tc.tile_pool(name="norm", bufs=2) as normp, \
             tc.tile_pool(name="ob", bufs=2) as obp, \
             tc.tile_pool(name="psS", bufs=3, space="PSUM") as psS, \
             tc.tile_pool(name="psAcc", bufs=2, space="PSUM") as psAcc:

            # ---- constants ----
            ident = const.tile([128, 128], F32)
            nc.sync.dma_start(ident[:], id_ap[:])
            tri = const.tile([128, 128], F32)
            nc.sync.dma_start(tri[:], tri_ap[:])
            bq_sb = const.tile([128, 4], F32)
            nc.sync.dma_start(bq_sb[:], bq_ap.rearrange("(t p) -> p t", p=128))
            bq_row = const.tile([1, DL], F32)
            nc.sync.dma_start(bq_row[:], bq_ap.rearrange("(a n) -> a n", a=1))
            bq_row_r = const.tile([1, DL], F32R)
            nc.vector.tensor_copy(bq_row_r[:], bq_row[:])
            ones_f = const.tile([128, 128], F32)
            nc.vector.memset(ones_f[:], 1.0)
            ones_r = const.tile([128, 128], F32R)
            nc.vector.tensor_copy(ones_r[:], ones_f[:])

            # ---- weights (cast to fp32r once) ----
            wq_r = persist.tile([128, JD, DL], F32R)
            for j in range(JD):
                st = stage.tile([128, DL], F32, tag="wst")
                nc.sync.dma_start(st[:], wq_ap[j * 128:(j + 1) * 128, :])
                nc.vector.tensor_copy(wq_r[:, j, :], st[:])
            wo_r = persist.tile([128, 4, D], F32R)
            for kt in range(4):
                st = stage.tile([128, D], F32, tag="wst2")
                nc.sync.dma_start(st[:], wo_ap[kt * 128:(kt + 1) * 128, :])
                nc.vector.tensor_copy(wo_r[:, kt, :], st[:])

            # persistent per-token-block tensors
            kT = [persist.tile([128, 4, 512], F32R, name=f"kT{i}", tag=f"kT{i}") for i in range(TB)]
            vv = [persist.tile([128, 4, NHL, HD + 1], F32R, name=f"vv{i}", tag=f"vv{i}") for i in range(TB)]

            q_tiles = [None] * TB

            def phase1(tb, rep):
                """transpose + project q,k,v for token block tb (512 tokens)."""
                for name in ("x_k", "x_v", "x_q"):
                    x_ap = x_aps[name]
                    xT = xtp.tile([128, JD, 512], F32R, tag="xt")
                    for sub in range(4):
                        xn = xnp.tile([128, D], F32, tag="xn")
                        r0 = tb * 512 + sub * 128
                        nc.sync.dma_start(xn[:], x_ap[r0:r0 + 128, :])
                        for jg in range(2):
                            pt = psS.tile([128, 512], F32, tag="sc")
                            for ji in range(4):
                                j = jg * 4 + ji
                                nc.tensor.transpose(
                                    pt[:, ji * 128:(ji + 1) * 128],
                                    xn[:, j * 128:(j + 1) * 128],
                                    ident[:],
                                )
                            nc.vector.tensor_copy(
                                xT[:, jg * 4:(jg + 1) * 4,
                                   sub * 128:(sub + 1) * 128],
                                pt[:].rearrange("p (j t) -> p j t", j=4),
                            )
                    if name == "x_v":
                        vt = vv[tb]
                        for sub in range(4):
                            pv = psS.tile([128, 512], F32, tag="sc")
                            for j in range(JD):
                                nc.tensor.matmul(
                                    pv[:],
                                    xT[:, j, sub * 128:(sub + 1) * 128],
                                    wq_r[:, j, :],
                                    start=(j == 0),
                                    stop=False,
                                )
                            nc.tensor.matmul(
                                pv[:],
                                ones_r[0:1, 0:128],
                                bq_row_r[:],
                                start=False,
                                stop=True,
                            )
                            nc.vector.tensor_copy(
                                vt[:, sub, :, 0:HD],
                                pv[:].rearrange("p (h d) -> p h d", h=NHL),
                            )
                        nc.vector.tensor_copy(
                            vt[:, :, :, HD],
                            ones_f[:, 0:32].rearrange("p (s h) -> p s h", s=4),
                        )
                    else:
                        if name == "x_q":
                            dest = qtp.tile([128, 4, 512], F32R, tag="qt")
                            q_tiles[tb] = dest
                        else:
                            dest = kT[tb]
                        for dt_ in range(4):
                            py = psS.tile([128, 512], F32, tag="sc")
                            for j in range(JD):
                                nc.tensor.matmul(
                                    py[:],
                                    wq_r[:, j, dt_ * 128:(dt_ + 1) * 128],
                                    xT[:, j, :],
                                    start=(j == 0),
                                    stop=(j == JD - 1),
                                )
                            nc.scalar.activation(
                                dest[:, dt_, :],
                                py[:],
                                AF.Identity,
                                bias=bq_sb[:, dt_:dt_ + 1],
                            )

            def attention(Q, rep):
                """attention + Wo for query block Q (512 tokens)."""
                attnT = atp.tile([128, 4, 512], F32R, tag="at")
                qtile = q_tiles[Q]
                nj = 4 * (Q + 1)
                for hp in range(4):
                    acc0 = psAcc.tile([128, 512], F32, tag="acc")
                    acc1 = psAcc.tile([128, 512], F32, tag="acc")

                    def emit_scores(j):
                        """scoresT pair + mask + exp for k-tile j; returns exp tile."""
                        tbj, sub = j // 4, j % 4
                        qoff = max(0, j * 128 - Q * 512)
                        ps = psS.tile([128, 2, 512], F32, tag="sc", name=f"ps{j}")
                        for hi, base in ((0, 0), (1, 64)):
                            nc.tensor.matmul(
                                ps[:, hi, qoff:],
                                kT[tbj][base:base + 64, hp,
                                        sub * 128:(sub + 1) * 128],
                                qtile[base:base + 64, hp, qoff:],
                                start=True,
                                stop=True,
                            )
                        if j * 128 >= Q * 512:  # diagonal k-tile: causal mask
                            for hi in range(2):
                                nc.vector.tensor_add(
                                    ps[:, hi, qoff:qoff + 128],
                                    ps[:, hi, qoff:qoff + 128],
                                    tri[:],
                                )
                        et = ep.tile([128, 2, 512], F32R, tag="exp", name=f"et{j}")
                        nc.scalar.activation(
                            et[:, :, qoff:], ps[:, :, qoff:], AF.Exp, scale=0.125
                        )
                        return et

                    def emit_attn(j, et):
                        tbj, sub = j // 4, j % 4
                        qoff = max(0, j * 128 - Q * 512)
                        for hi, acc in ((0, acc0), (1, acc1)):
                            nc.tensor.matmul(
                                acc[0:65, qoff:],
                                vv[tbj][:, sub, hp * 2 + hi, :],
                                et[:, hi, qoff:],
                                start=(j == 0),
                                stop=(j == nj - 1),
                            )

                    # software pipeline: scores/exp run one k-tile ahead of
                    # the accumulating attn matmuls so the in-order PE
                    # stream never head-blocks on the ACT exp.
                    et_prev = emit_scores(0)
                    for j in range(1, nj):
                        et_cur = emit_scores(j)
                        emit_attn(j - 1, et_prev)
                        et_prev = et_cur
                    emit_attn(nj - 1, et_prev)
                    for hi, acc in ((0, acc0), (1, acc1)):
                        # sum row (f32r) -> rank-1 broadcast -> wide recip
                        sr = normp.tile([128, 512], F32R, tag="sr")
                        nc.vector.tensor_copy(sr[64:65, :], acc[64:65, :])
                        pb = psS.tile([128, 512], F32, tag="sc")
                        nc.tensor.matmul(
                            pb[0:64, :],
                            ones_r[64:65, 0:64],
                            sr[64:65, :],
                            start=True,
                            stop=True,
                        )
                        rb = normp.tile([64, 512], F32, tag="rb")
                        nc.vector.reciprocal(rb[:], pb[0:64, :])
                        nc.vector.tensor_mul(
                            attnT[hi * 64:(hi + 1) * 64, hp, :],
                            acc[0:64, :],
                            rb[:],
                        )
                # Wo projection for this token block
                for st_ in range(4):
                    ob = obp.tile([128, D], F32, tag="ob")
                    for nh in range(2):
                        po = psS.tile([128, 512], F32, tag="sc")
                        for kt in range(4):
                            nc.tensor.matmul(
                                po[:],
                                attnT[:, kt, st_ * 128:(st_ + 1) * 128],
                                wo_r[:, kt, nh * 512:(nh + 1) * 512],
                                start=(kt == 0),
                                stop=(kt == 3),
                            )
                        nc.vector.tensor_copy(ob[:, nh * 512:(nh + 1) * 512], po[:])
                    r0 = Q * 512 + st_ * 128
                    nc.sync.dma_start(out_ap[r0:r0 + 128, :], ob[:])

            if mode == "full":
                for rep in range(repeat):
                    for tb in range(TB):
                        phase1(tb, rep)
                    for Q in range(TB):
                        attention(Q, rep)
            elif mode == "p1":
                for rep in range(repeat):
                    for tb in range(TB):
                        phase1(tb, rep)
                for Q in range(TB):
                    attention(Q, 0)
            elif mode == "attn":
                for tb in range(TB):
                    phase1(tb, 0)
                for rep in range(repeat):
                    for Q in range(TB):
                        attention(Q, rep)

    nc.compile()
    return nc


_BUILD_CACHE = {}


def _get(repeat=1, mode="full"):
    key = (repeat, mode)
    if key not in _BUILD_CACHE:
        _BUILD_CACHE[key] = build(repeat, mode)
    return _BUILD_CACHE[key]


def make_in_maps(q, k, v, Wq, bq, Wo, bo):
    tri = np.where(
        np.arange(128)[:, None] <= np.arange(128)[None, :], 0.0, NEG
    ).astype(np.float32)
    ident = np.eye(128, dtype=np.float32)
    in_maps = []
    for c in range(8):
        b, g = c // 2, c % 2
        sl = slice(g * DL, (g + 1) * DL)
        in_maps.append({
            "x_q": np.ascontiguousarray(q[b]),
            "x_k": np.ascontiguousarray(k[b]),
            "x_v": np.ascontiguousarray(v[b]),
            "wq": np.ascontiguousarray(Wq[:, sl]),
            "bq": np.ascontiguousarray(bq[sl]),
            "wo": np.ascontiguousarray(Wo[sl, :]),
            "tri": tri,
            "ident": ident,
        })
    return in_maps


def kernel(q, k, v, Wq, bq, Wo, bo):
    q, k, v, Wq, bq, Wo, bo = (
        np.asarray(a, dtype=np.float32) for a in (q, k, v, Wq, bq, Wo, bo)
    )
    nc = _get(1)
    in_maps = make_in_maps(q, k, v, Wq, bq, Wo, bo)
    res = run_bass_kernel_spmd(nc, in_maps, list(range(8)))
    B = q.shape[0]
    out = np.empty((B, S, D), dtype=np.float32)
    for b in range(B):
        out[b] = res.results[2 * b]["out"] + res.results[2 * b + 1]["out"] + bo
    return out
